# revision 4
# baseline (speedup 1.0000x reference)
"""Trainium2 Bass kernel for nn_CrossModalAttention (B=4, S=2048, H=2048, single head).

Sharding: 8 cores = 4 batches x 2 sequence halves. Each core computes the full
attention output for its (batch, query-half): it projects its 1024 query rows
and (duplicated across the pair) the full 2048 K/V rows of its batch element.

Device-side dataflow keeps every activation TRANSPOSED ([feature, seq]) so all
matmuls contract over the SBUF partition dim with no on-device transposes:

  P0  RoPE (DVE):    qrotT/krotT = qT*cos + rot_half(qT)*sin   -> DRAM staging
  P1  v    = value @ WvT            (lhsT = vT strips,  rhs = WvT resident)
  P2  kT'  = (WkT.T @ krotT) + bk   (lhsT = WkT tiles,  rhs = krotT chunks)
  P3  qT'  = (WqT.T @ qrotT) + bq
  P4  expT = exp(scale * kT'.T@qT') ; den = ones.T @ expT  (softmax w/o max-sub;
      scores are bounded ~|4| so exp cannot overflow)
  P5  ctxT = (v.T-strips @ expT) * (1/den bcast) + bv      (softmax normalization
      and the V bias are folded into the PSUM eviction; sum(probs)=1 makes the
      bias fold exact)
  P6  outT = (WoT.T @ ctxT) + bo  -> DRAM

All matmul operands are float32r (FP22 multiply, fp32 accumulate): full PE rate
at moving-dim >= 256 with ~2^-13 relative precision.
"""

import sys

for _p in ("/opt/trn_rl_repo",):
    if _p not in sys.path:
        sys.path.append(_p)

import numpy as np

B, S, H = 4, 2048, 2048
P = 128
HO = H // P            # 16 h-tiles
OT = H // P            # 16 o-tiles
SKT = S // P           # 16 key-position tiles
SQ = S // 2            # 1024 query rows per core
SK = S                 # 2048 key rows
NC_ = 8
SCALE = 1.0 / float(np.sqrt(H))

_PROG = None  # cached compiled program
_TRACE = False  # set True (with an NTFF hook installed) to profile the run
LAST_RES = None  # BassKernelResults of the most recent kernel() call


def _emit(nc, tile, mybir):
    F32 = mybir.dt.float32
    F32R = mybir.dt.float32r
    Exp = mybir.ActivationFunctionType.Exp
    Ident = mybir.ActivationFunctionType.Identity

    dram = {}
    def din(name, shape, dt=F32):
        dram[name] = nc.dram_tensor(name, list(shape), dt, kind="ExternalInput").ap()
    din("qT", (H, SQ)); din("kT", (H, SK))
    din("vT", (H, SK), F32R)
    din("cos_q", (H // 2, SQ)); din("sin_q", (H // 2, SQ))
    din("cos_k", (H // 2, SK)); din("sin_k", (H // 2, SK))
    din("wq", (H, H), F32R); din("wk", (H, H), F32R)
    din("wv", (H, H), F32R); din("wo", (H, H), F32R)
    din("bq", (H,)); din("bk", (H,)); din("bv", (H,)); din("bo", (H,))
    din("ones_col", (P, 1), F32R); din("ones_row", (1, P), F32R)
    outT = nc.dram_tensor("outT", [H, SQ], F32, kind="ExternalOutput").ap()

    def ld_strip(pool, src2d, col0, width, tag):
        """DMA a [rows, width] column strip of a DRAM matrix into [P, rows/P, width]."""
        rows = src2d.shape[0]
        t = pool.tile([P, rows // P, width], src2d.dtype, name=tag)
        nc.sync.dma_start(
            out=t[:],
            in_=src2d[:, col0:col0 + width].rearrange("(o p) s -> p o s", p=P),
        )
        return t

    with tile.TileContext(nc) as tc:
        from contextlib import ExitStack
        with ExitStack() as ctx:
            glob = ctx.enter_context(tc.tile_pool(name="glob", bufs=1))
            dstage = ctx.enter_context(tc.tile_pool(name="dram", bufs=1, space="DRAM"))
            psum = ctx.enter_context(tc.tile_pool(name="psum", bufs=4, space="PSUM"))

            # staging buffers in HBM
            qrot_st = dstage.tile([H, SQ], F32R, name="qrot_st")
            krot_st = dstage.tile([H, SK], F32R, name="krot_st")
            v_st = dstage.tile([SK, H], F32R, name="v_st")
            kTp_st = dstage.tile([H, SK], F32R, name="kTp_st")
            qTp_st = dstage.tile([H, SQ], F32R, name="qTp_st")

            # biases as [P, 16] (+ ones helpers)
            bias_sb = {}
            for bn in ("bq", "bk", "bv", "bo"):
                bt = glob.tile([P, OT], F32, name=f"{bn}_sb")
                nc.sync.dma_start(out=bt[:], in_=dram[bn].rearrange("(t p) -> p t", p=P))
                bias_sb[bn] = bt
            ones_col = glob.tile([P, 1], F32R, name="ones_col")
            nc.sync.dma_start(out=ones_col[:], in_=dram["ones_col"])
            ones_row = glob.tile([1, P], F32R, name="ones_row")
            nc.sync.dma_start(out=ones_row[:], in_=dram["ones_row"])

            # ---------------- P1: v = value @ WvT  (+ P0 RoPE staging overlapped)
            def rope_stage(src, cos_ap, sin_ap, dst, ncols, cw, pool):
                for c0 in range(0, ncols, cw):
                    x = pool.tile([P, HO, cw], F32, name="rope_in")
                    nc.sync.dma_start(out=x[:], in_=src[:, c0:c0 + cw].rearrange("(o p) s -> p o s", p=P))
                    cs = pool.tile([P, HO // 2, cw], F32, name="rope_cos")
                    nc.sync.dma_start(out=cs[:], in_=cos_ap[:, c0:c0 + cw].rearrange("(o p) s -> p o s", p=P))
                    sn = pool.tile([P, HO // 2, cw], F32, name="rope_sin")
                    nc.sync.dma_start(out=sn[:], in_=sin_ap[:, c0:c0 + cw].rearrange("(o p) s -> p o s", p=P))
                    r = pool.tile([P, HO, cw], F32R, name="rope_out")
                    tmp = pool.tile([P, cw], F32, name="rope_tmp")
                    for i in range(HO):
                        j = i & (HO // 2 - 1)
                        nc.vector.tensor_mul(r[:, i, :], x[:, i, :], cs[:, j, :])
                        nc.vector.tensor_mul(tmp[:], x[:, i ^ (HO // 2), :], sn[:, j, :])
                        if i < HO // 2:
                            nc.vector.tensor_sub(r[:, i, :], r[:, i, :], tmp[:])
                        else:
                            nc.vector.tensor_add(r[:, i, :], r[:, i, :], tmp[:])
                    nc.sync.dma_start(
                        out=dst[:, c0:c0 + cw].rearrange("(o p) s -> p o s", p=P),
                        in_=r[:],
                    )

            with tc.tile_pool(name="p1w", bufs=1) as p1w:
                wv_sb = p1w.tile([P, HO, H], F32R, name="wv_sb")
                nc.sync.dma_start(out=wv_sb[:], in_=dram["wv"].rearrange("(o p) s -> p o s", p=P))
                with tc.tile_pool(name="p1s", bufs=2) as p1s, \
                     tc.tile_pool(name="p1e", bufs=3) as p1e:
                    for st in range(SKT):
                        vstrip = ld_strip(p1s, dram["vT"], st * P, P, "vT_strip")
                        for oc in range(4):
                            ps = psum.tile([P, 512], F32, name="ps_mm")
                            for h in range(HO):
                                nc.tensor.matmul(
                                    ps[:], vstrip[:, h, :], wv_sb[:, h, oc * 512:(oc + 1) * 512],
                                    start=(h == 0), stop=(h == HO - 1))
                            ev = p1e.tile([P, 512], F32R, name="p1_ev")
                            nc.scalar.copy(ev[:], ps[:])
                            nc.sync.dma_start(
                                out=v_st[st * P:(st + 1) * P, oc * 512:(oc + 1) * 512],
                                in_=ev[:])
                    with tc.tile_pool(name="p0k", bufs=2) as p0k:
                        rope_stage(dram["kT"], dram["cos_k"], dram["sin_k"], krot_st, SK, 128, p0k)

            # ---------------- P2: kT' = WkT.T @ krotT + bk   (+ P0q overlapped)
            def proj(w_ap, rot_st, dst_st, bias_t, ncols, cw, tag):
                with tc.tile_pool(name=f"{tag}w", bufs=1) as pw:
                    w_sb = pw.tile([P, HO, H], F32R, name=f"{tag}_w")
                    nc.sync.dma_start(out=w_sb[:], in_=w_ap.rearrange("(o p) s -> p o s", p=P))
                    with tc.tile_pool(name=f"{tag}s", bufs=2) as psb, \
                         tc.tile_pool(name=f"{tag}e", bufs=3) as peb:
                        for c0 in range(0, ncols, cw):
                            rch = psb.tile([P, HO, cw], F32R, name=f"{tag}_rch")
                            nc.sync.dma_start(
                                out=rch[:],
                                in_=rot_st[:, c0:c0 + cw].rearrange("(o p) s -> p o s", p=P))
                            for ot in range(OT):
                                ps = psum.tile([P, 512], F32, name="ps_mm")[:, :cw]
                                for h in range(HO):
                                    nc.tensor.matmul(
                                        ps, w_sb[:, h, ot * P:(ot + 1) * P], rch[:, h, :],
                                        start=(h == 0), stop=(h == HO - 1))
                                ev = peb.tile([P, 512], F32R, name=f"{tag}_ev")[:, :cw]
                                nc.scalar.activation(ev, ps, Ident, bias=bias_t[:, ot:ot + 1])
                                nc.sync.dma_start(
                                    out=dst_st[ot * P:(ot + 1) * P, c0:c0 + cw], in_=ev)

            # P2 needs krot fully staged; its weight DMA + first chunk can still
            # overlap P1's tail. P0q runs (DVE) under P2's PE work.
            with tc.tile_pool(name="p2outer", bufs=1):
                pass
            # emit P2 with a nested P0q
            with tc.tile_pool(name="p2w", bufs=1) as pw2:
                wk_sb = pw2.tile([P, HO, H], F32R, name="wk_sb")
                nc.sync.dma_start(out=wk_sb[:], in_=dram["wk"].rearrange("(o p) s -> p o s", p=P))
                with tc.tile_pool(name="p2s", bufs=2) as p2s, \
                     tc.tile_pool(name="p2e", bufs=3) as p2e:
                    CW2 = 256
                    for c0 in range(0, SK, CW2):
                        rch = p2s.tile([P, HO, CW2], F32R, name="p2_rch")
                        nc.sync.dma_start(
                            out=rch[:],
                            in_=krot_st[:, c0:c0 + CW2].rearrange("(o p) s -> p o s", p=P))
                        for ot in range(OT):
                            ps = psum.tile([P, 512], F32, name="ps_mm")[:, :CW2]
                            for h in range(HO):
                                nc.tensor.matmul(
                                    ps, wk_sb[:, h, ot * P:(ot + 1) * P], rch[:, h, :],
                                    start=(h == 0), stop=(h == HO - 1))
                            ev = p2e.tile([P, 512], F32R, name="p2_ev")[:, :CW2]
                            nc.scalar.activation(ev, ps, Ident, bias=bias_sb["bk"][:, ot:ot + 1])
                            nc.sync.dma_start(
                                out=kTp_st[ot * P:(ot + 1) * P, c0:c0 + CW2], in_=ev)
                with tc.tile_pool(name="p0q", bufs=2) as p0q:
                    rope_stage(dram["qT"], dram["cos_q"], dram["sin_q"], qrot_st, SQ, 128, p0q)

            # ---------------- P3: qT' = WqT.T @ qrotT + bq
            proj(dram["wq"], qrot_st, qTp_st, bias_sb["bq"], SQ, 512, "p3")

            # ---------------- P4/P5/P6
            with tc.tile_pool(name="expT", bufs=1) as expT_pool:
                expT = expT_pool.tile([P, SKT, SQ], F32R, name="expT")
                recip_bc = expT_pool.tile([P, SQ], F32, name="recip_bc")
                with tc.tile_pool(name="p4q", bufs=1) as p4q, \
                     tc.tile_pool(name="p4s", bufs=2) as p4s, \
                     tc.tile_pool(name="p4den", bufs=2, space="PSUM") as p4den, \
                     tc.tile_pool(name="p4m", bufs=2) as p4m:
                    qTp_sb = p4q.tile([P, OT, SQ], F32R, name="qTp_sb")
                    nc.sync.dma_start(out=qTp_sb[:], in_=qTp_st.rearrange("(o p) s -> p o s", p=P))
                    den_ps = [p4den.tile([1, 512], F32, name=f"den{c}") for c in range(2)]
                    for st in range(SKT):
                        kstrip = ld_strip(p4s, kTp_st, st * P, P, "kTp_strip")
                        for c in range(2):
                            ps = psum.tile([P, 512], F32, name="ps_mm")
                            for o in range(OT):
                                nc.tensor.matmul(
                                    ps[:], kstrip[:, o, :], qTp_sb[:, o, c * 512:(c + 1) * 512],
                                    start=(o == 0), stop=(o == OT - 1))
                            esl = expT[:, st, c * 512:(c + 1) * 512]
                            nc.scalar.activation(esl, ps[:], Exp, scale=SCALE)
                            nc.tensor.matmul(den_ps[c][:], ones_col[:], esl,
                                             start=(st == 0), stop=(st == SKT - 1))
                    for c in range(2):
                        rec = p4m.tile([1, 512], F32R, name="rec")
                        with nc.allow_low_precision("fp32r is 4-byte; feeds PE broadcast"):
                            nc.vector.reciprocal(rec[:], den_ps[c][:])
                        bc = psum.tile([P, 512], F32, name="ps_mm")
                        nc.tensor.matmul(bc[:], ones_row[:], rec[:], start=True, stop=True)
                        nc.vector.tensor_copy(recip_bc[:, c * 512:(c + 1) * 512], bc[:])

                with tc.tile_pool(name="ctxT", bufs=1) as ctx_pool:
                    ctxT = ctx_pool.tile([P, OT, SQ], F32R, name="ctxT")
                    with tc.tile_pool(name="p5s", bufs=2) as p5s:
                        for ot in range(OT):
                            vstrip = ld_strip(p5s, v_st, ot * P, P, "v_strip")
                            for c in range(2):
                                ps = psum.tile([P, 512], F32, name="ps_mm")
                                for st in range(SKT):
                                    nc.tensor.matmul(
                                        ps[:], vstrip[:, st, :], expT[:, st, c * 512:(c + 1) * 512],
                                        start=(st == 0), stop=(st == SKT - 1))
                                csl = ctxT[:, ot, c * 512:(c + 1) * 512]
                                nc.vector.tensor_mul(csl, ps[:], recip_bc[:, c * 512:(c + 1) * 512])
                                nc.vector.tensor_scalar_add(csl, csl, bias_sb["bv"][:, ot:ot + 1])

                    with tc.tile_pool(name="p6s", bufs=2) as p6s, \
                         tc.tile_pool(name="p6o", bufs=2) as p6o:
                        for mt in range(OT):
                            wstrip = ld_strip(p6s, dram["wo"], mt * P, P, "wo_strip")
                            outt = p6o.tile([P, SQ], F32, name="outt")
                            for c in range(2):
                                ps = psum.tile([P, 512], F32, name="ps_mm")
                                for o in range(OT):
                                    nc.tensor.matmul(
                                        ps[:], wstrip[:, o, :], ctxT[:, o, c * 512:(c + 1) * 512],
                                        start=(o == 0), stop=(o == OT - 1))
                                nc.scalar.activation(outt[:, c * 512:(c + 1) * 512], ps[:],
                                                     Ident, bias=bias_sb["bo"][:, mt:mt + 1])
                            nc.sync.dma_start(out=outT[mt * P:(mt + 1) * P, :], in_=outt[:])
    return nc


def _build():
    global _PROG
    if _PROG is not None:
        return _PROG
    import concourse.bass as bass  # noqa: F401
    import concourse.mybir as mybir
    import concourse.tile as tile
    from concourse import bacc

    nc = bacc.Bacc("TRN2", target_bir_lowering=False, debug=False)
    _emit(nc, tile, mybir)
    nc.compile()
    _PROG = nc
    return nc


def _rope_tables():
    inv_freq = 1.0 / (10000.0 ** (np.arange(0, H, 2, dtype=np.float32) / H))
    t = np.arange(S, dtype=np.float32)
    freqs = np.outer(t, inv_freq).astype(np.float32)      # [S, H/2]
    cosT = np.ascontiguousarray(np.cos(freqs).T)          # [H/2, S]
    sinT = np.ascontiguousarray(np.sin(freqs).T)
    return cosT, sinT


def kernel(**inputs):
    nc = _build()
    from concourse.bass_utils import run_bass_kernel_spmd

    q = np.asarray(inputs["query"], dtype=np.float32)
    k = np.asarray(inputs["key"], dtype=np.float32)
    v = np.asarray(inputs["value"], dtype=np.float32)
    cosT, sinT = _rope_tables()
    wT = {n: np.ascontiguousarray(np.asarray(inputs[n], dtype=np.float32).T)
          for n in ("Wq", "Wk", "Wv", "Wo")}
    bias = {n: np.ascontiguousarray(np.asarray(inputs[n], dtype=np.float32))
            for n in ("bq", "bk", "bv", "bo")}
    ones_col = np.ones((P, 1), np.float32)
    ones_row = np.ones((1, P), np.float32)

    in_maps = []
    for c in range(NC_):
        b, half = divmod(c, 2)
        sl = slice(half * SQ, (half + 1) * SQ)
        qT = np.ascontiguousarray(q[b].T[:, sl])
        in_maps.append({
            "qT": qT,
            "kT": np.ascontiguousarray(k[b].T),
            "vT": np.ascontiguousarray(v[b].T),
            "cos_q": np.ascontiguousarray(cosT[:, sl]),
            "sin_q": np.ascontiguousarray(sinT[:, sl]),
            "cos_k": cosT, "sin_k": sinT,
            "wq": wT["Wq"], "wk": wT["Wk"], "wv": wT["Wv"], "wo": wT["Wo"],
            "bq": bias["bq"], "bk": bias["bk"], "bv": bias["bv"], "bo": bias["bo"],
            "ones_col": ones_col, "ones_row": ones_row,
        })

    res = run_bass_kernel_spmd(nc, in_maps, core_ids=list(range(NC_)), trace=_TRACE)
    global LAST_RES
    LAST_RES = res
    out = np.empty((B, S, H), np.float32)
    for c in range(NC_):
        b, half = divmod(c, 2)
        out[b, half * SQ:(half + 1) * SQ, :] = res.results[c]["outT"].T
    return out


# revision 5
# speedup vs baseline: 1.2247x; 1.2247x over previous
"""Trainium2 Bass kernel for nn_CrossModalAttention (B=4, S=2048, H=2048, single head).

Sharding: 8 cores = 4 batches x 2 sequence halves. Each core computes the full
attention output for its (batch, query-half): it projects its 1024 query rows
and (duplicated across the pair) the full 2048 K/V rows of its batch element.

Device-side dataflow keeps every activation TRANSPOSED ([feature, seq]) so all
matmuls contract over the SBUF partition dim with no on-device transposes:

  A   v = value @ WvT   (WvT resident in 2 o-halves so the 2nd half's load
      hides under the 1st half's matmuls)  -> v staged to HBM
      + concurrently: Q-RoPE on DVE (3D tensor ops) -> qrotT staged to HBM
  B   kT' = (WkT.T @ krotT) + bk with K-RoPE fused inline per chunk (no HBM
      round-trip for krotT)               -> kT' staged to HBM
  C   qT' = (WqT.T @ qrotT) + bq  (WqT in 2 halves)  -> qT' staged to HBM
  D   expT = exp(scale * kT'.T @ qT') ; den = ones.T @ expT  (softmax without
      max-subtraction; scores are bounded ~|4| so exp cannot overflow)
  E   ctxT = (v.T-strips @ expT) * (1/den bcast) + bv   (softmax normalization
      and the V bias folded into the PSUM eviction; sum(probs)=1 makes the
      bias fold exact)
  F   outT = (WoT.T @ ctxT) + bo  -> HBM

All matmul operands are float32r (FP22 multiply, fp32 accumulate): full PE rate
at moving-dim >= 256 with ~2^-13 relative precision. RoPE tables are fp16.
"""

import sys

for _p in ("/opt/trn_rl_repo",):
    if _p not in sys.path:
        sys.path.append(_p)

import numpy as np

B, S, H = 4, 2048, 2048
P = 128
HO = H // P            # 16 h-tiles
OT = H // P            # 16 o-tiles
SKT = S // P           # 16 key-position tiles
SQ = S // 2            # 1024 query rows per core
SK = S                 # 2048 key rows
NC_ = 8
SCALE = 1.0 / float(np.sqrt(H))

_PROG = None  # cached compiled program
_TRACE = False  # set True (with an NTFF hook installed) to profile the run
LAST_RES = None  # BassKernelResults of the most recent kernel() call


def _emit(nc, tile, mybir):
    F32 = mybir.dt.float32
    F16 = mybir.dt.float16
    F32R = mybir.dt.float32r
    Exp = mybir.ActivationFunctionType.Exp
    Ident = mybir.ActivationFunctionType.Identity

    dram = {}
    def din(name, shape, dt=F32):
        dram[name] = nc.dram_tensor(name, list(shape), dt, kind="ExternalInput").ap()
    din("qT", (H, SQ)); din("kT", (H, SK))
    din("vT", (H, SK), F32R)
    din("cos_q", (H // 2, SQ), F16); din("sin_q", (H // 2, SQ), F16)
    din("cos_k", (H // 2, SK), F16); din("sin_k", (H // 2, SK), F16)
    din("wq", (H, H), F32R); din("wk", (H, H), F32R)
    din("wv", (H, H), F32R); din("wo", (H, H), F32R)
    din("bq", (H,)); din("bk", (H,)); din("bv", (H,)); din("bo", (H,))
    din("ones_col", (P, 1), F32R); din("ones_row", (1, P), F32R)
    outT = nc.dram_tensor("outT", [H, SQ], F32, kind="ExternalOutput").ap()

    def strip_ap(src2d, col0, width):
        return src2d[:, col0:col0 + width].rearrange("(o p) s -> p o s", p=P)

    def ld_strip(pool, src2d, col0, width, tag):
        rows = src2d.shape[0]
        t = pool.tile([P, rows // P, width], src2d.dtype, name=tag)
        nc.sync.dma_start(out=t[:], in_=strip_ap(src2d, col0, width))
        return t

    def rope_chunk(pool, xsrc, cos_ap, sin_ap, c0, cw, tag):
        """Load [H, cw] chunk of xsrc + tables, return roped [P, HO, cw] f32r tile.

        rot_half: lo half uses x_lo*cos - x_hi*sin; hi half x_hi*cos + x_lo*sin.
        """
        HH = HO // 2
        x = pool.tile([P, HO, cw], F32, name=f"{tag}_in")
        nc.sync.dma_start(out=x[:], in_=strip_ap(xsrc, c0, cw))
        cs = pool.tile([P, HH, cw], F16, name=f"{tag}_cos")
        nc.sync.dma_start(out=cs[:], in_=strip_ap(cos_ap, c0, cw))
        sn = pool.tile([P, HH, cw], F16, name=f"{tag}_sin")
        nc.sync.dma_start(out=sn[:], in_=strip_ap(sin_ap, c0, cw))
        r = pool.tile([P, HO, cw], F32R, name=f"{tag}_out")
        tmp = pool.tile([P, HH, cw], F32, name=f"{tag}_tmp")
        lo, hi = slice(0, HH), slice(HH, HO)
        nc.vector.tensor_mul(r[:, lo, :], x[:, lo, :], cs[:])
        nc.vector.tensor_mul(tmp[:], x[:, hi, :], sn[:])
        nc.vector.tensor_sub(r[:, lo, :], r[:, lo, :], tmp[:])
        nc.vector.tensor_mul(r[:, hi, :], x[:, hi, :], cs[:])
        nc.vector.tensor_mul(tmp[:], x[:, lo, :], sn[:])
        nc.vector.tensor_add(r[:, hi, :], r[:, hi, :], tmp[:])
        return r

    with tile.TileContext(nc) as tc:
        from contextlib import ExitStack
        with ExitStack() as ctx:
            glob = ctx.enter_context(tc.tile_pool(name="glob", bufs=1))
            dstage = ctx.enter_context(tc.tile_pool(name="dram", bufs=1, space="DRAM"))
            psum = ctx.enter_context(tc.tile_pool(name="psum", bufs=4, space="PSUM"))

            # staging buffers in HBM
            qrot_st = dstage.tile([H, SQ], F32R, name="qrot_st")
            v_st = dstage.tile([SK, H], F32R, name="v_st")
            kTp_st = dstage.tile([H, SK], F32R, name="kTp_st")
            qTp_st = dstage.tile([H, SQ], F32R, name="qTp_st")

            # ---- Phase A: v-proj (wv halves) + q-rope staging, concurrent ----
            with tc.tile_pool(name="wvh0", bufs=1) as wvh0p, \
                 tc.tile_pool(name="wvh1", bufs=1) as wvh1p, \
                 tc.tile_pool(name="p1s", bufs=2) as p1s, \
                 tc.tile_pool(name="p1e", bufs=2) as p1e, \
                 tc.tile_pool(name="qrope", bufs=2) as qrope:
                wv_h = [wvh0p.tile([P, HO, H // 2], F32R, name="wv_h0"),
                        wvh1p.tile([P, HO, H // 2], F32R, name="wv_h1")]
                for hf in range(2):
                    nc.sync.dma_start(out=wv_h[hf][:],
                                      in_=strip_ap(dram["wv"], hf * (H // 2), H // 2))

                # biases as [P, 16] + ones helpers
                bias_sb = {}
                for bn in ("bq", "bk", "bv", "bo"):
                    bt = glob.tile([P, OT], F32, name=f"{bn}_sb")
                    nc.sync.dma_start(out=bt[:], in_=dram[bn].rearrange("(t p) -> p t", p=P))
                    bias_sb[bn] = bt
                ones_col = glob.tile([P, 1], F32R, name="ones_col")
                nc.sync.dma_start(out=ones_col[:], in_=dram["ones_col"])
                ones_row = glob.tile([1, P], F32R, name="ones_row")
                nc.sync.dma_start(out=ones_row[:], in_=dram["ones_row"])
                recip_bc = glob.tile([P, SQ], F32, name="recip_bc")

                for hf in range(2):
                    for st in range(SKT):
                        vstrip = ld_strip(p1s, dram["vT"], st * P, P, "vT_strip")
                        for oc in range(2):
                            ps = psum.tile([P, 512], F32, name="ps_mm")
                            for h in range(HO):
                                nc.tensor.matmul(
                                    ps[:], vstrip[:, h, :],
                                    wv_h[hf][:, h, oc * 512:(oc + 1) * 512],
                                    start=(h == 0), stop=(h == HO - 1))
                            ev = p1e.tile([P, 512], F32R, name="p1_ev")
                            nc.scalar.copy(ev[:], ps[:])
                            nc.sync.dma_start(
                                out=v_st[st * P:(st + 1) * P,
                                         (hf * 2 + oc) * 512:(hf * 2 + oc + 1) * 512],
                                in_=ev[:])

                # q-rope -> qrot_st (DVE work; overlaps the PE-heavy loop above)
                for c0 in range(0, SQ, 128):
                    r = rope_chunk(qrope, dram["qT"], dram["cos_q"], dram["sin_q"],
                                   c0, 128, "qr")
                    nc.sync.dma_start(out=strip_ap(qrot_st, c0, 128), in_=r[:])

            # ---- Phase B: k-proj with fused k-rope (full WkT resident) ----
            CW2 = 256
            with tc.tile_pool(name="p2w", bufs=1) as pw2, \
                 tc.tile_pool(name="p2in", bufs=1) as p2in, \
                 tc.tile_pool(name="p2tab", bufs=1) as p2tab, \
                 tc.tile_pool(name="p2r", bufs=2) as p2r, \
                 tc.tile_pool(name="p2e", bufs=3) as p2e:
                wk_sb = pw2.tile([P, HO, H], F32R, name="wk_sb")
                nc.sync.dma_start(out=wk_sb[:], in_=dram["wk"].rearrange("(o p) s -> p o s", p=P))
                for c0 in range(0, SK, CW2):
                    HH = HO // 2
                    x = p2in.tile([P, HO, CW2], F32, name="kr_in")
                    nc.sync.dma_start(out=x[:], in_=strip_ap(dram["kT"], c0, CW2))
                    cs = p2tab.tile([P, HH, CW2], F16, name="kr_cos")
                    nc.sync.dma_start(out=cs[:], in_=strip_ap(dram["cos_k"], c0, CW2))
                    sn = p2tab.tile([P, HH, CW2], F16, name="kr_sin")
                    nc.sync.dma_start(out=sn[:], in_=strip_ap(dram["sin_k"], c0, CW2))
                    r = p2r.tile([P, HO, CW2], F32R, name="kr_out")
                    tmp = p2r.tile([P, HH, CW2], F32, name="kr_tmp")
                    lo, hi = slice(0, HH), slice(HH, HO)
                    nc.vector.tensor_mul(r[:, lo, :], x[:, lo, :], cs[:])
                    nc.vector.tensor_mul(tmp[:], x[:, hi, :], sn[:])
                    nc.vector.tensor_sub(r[:, lo, :], r[:, lo, :], tmp[:])
                    nc.vector.tensor_mul(r[:, hi, :], x[:, hi, :], cs[:])
                    nc.vector.tensor_mul(tmp[:], x[:, lo, :], sn[:])
                    nc.vector.tensor_add(r[:, hi, :], r[:, hi, :], tmp[:])
                    for ot in range(OT):
                        ps = psum.tile([P, 512], F32, name="ps_mm")[:, :CW2]
                        for h in range(HO):
                            nc.tensor.matmul(
                                ps, wk_sb[:, h, ot * P:(ot + 1) * P], r[:, h, :],
                                start=(h == 0), stop=(h == HO - 1))
                        ev = p2e.tile([P, CW2], F32R, name="p2_ev")
                        nc.scalar.activation(ev[:], ps, Ident, bias=bias_sb["bk"][:, ot:ot + 1])
                        nc.sync.dma_start(
                            out=kTp_st[ot * P:(ot + 1) * P, c0:c0 + CW2], in_=ev[:])

            # ---- Phase C: q-proj (wq halves, qrot from HBM) ----
            with tc.tile_pool(name="wqh0", bufs=1) as wqh0p, \
                 tc.tile_pool(name="wqh1", bufs=1) as wqh1p, \
                 tc.tile_pool(name="p3in", bufs=2) as p3in, \
                 tc.tile_pool(name="p3e", bufs=3) as p3e:
                wq_h = [wqh0p.tile([P, HO, H // 2], F32R, name="wq_h0"),
                        wqh1p.tile([P, HO, H // 2], F32R, name="wq_h1")]
                for hf in range(2):
                    nc.sync.dma_start(out=wq_h[hf][:],
                                      in_=strip_ap(dram["wq"], hf * (H // 2), H // 2))
                for hf in range(2):
                    for c0 in range(0, SQ, 512):
                        rch = p3in.tile([P, HO, 512], F32R, name="p3_rch")
                        nc.sync.dma_start(out=rch[:], in_=strip_ap(qrot_st, c0, 512))
                        for otl in range(OT // 2):
                            ot = hf * (OT // 2) + otl
                            ps = psum.tile([P, 512], F32, name="ps_mm")
                            for h in range(HO):
                                nc.tensor.matmul(
                                    ps[:], wq_h[hf][:, h, otl * P:(otl + 1) * P],
                                    rch[:, h, :],
                                    start=(h == 0), stop=(h == HO - 1))
                            ev = p3e.tile([P, 512], F32R, name="p3_ev")
                            nc.scalar.activation(ev[:], ps[:], Ident,
                                                 bias=bias_sb["bq"][:, ot:ot + 1])
                            nc.sync.dma_start(
                                out=qTp_st[ot * P:(ot + 1) * P, c0:c0 + 512], in_=ev[:])

            # ---- Phase D: scores.T -> exp -> den ----
            with tc.tile_pool(name="expT", bufs=1) as expT_pool:
                expT = expT_pool.tile([P, SKT, SQ], F32R, name="expT")
                with tc.tile_pool(name="p4q", bufs=1) as p4q, \
                     tc.tile_pool(name="p4s", bufs=2) as p4s, \
                     tc.tile_pool(name="p4den", bufs=2, space="PSUM") as p4den, \
                     tc.tile_pool(name="p4m", bufs=2) as p4m:
                    qTp_sb = p4q.tile([P, OT, SQ], F32R, name="qTp_sb")
                    nc.sync.dma_start(out=qTp_sb[:], in_=qTp_st.rearrange("(o p) s -> p o s", p=P))
                    den_ps = [p4den.tile([1, 512], F32, name=f"den{c}") for c in range(2)]
                    for st in range(SKT):
                        kstrip = ld_strip(p4s, kTp_st, st * P, P, "kTp_strip")
                        for c in range(2):
                            ps = psum.tile([P, 512], F32, name="ps_mm")
                            for o in range(OT):
                                nc.tensor.matmul(
                                    ps[:], kstrip[:, o, :], qTp_sb[:, o, c * 512:(c + 1) * 512],
                                    start=(o == 0), stop=(o == OT - 1))
                            esl = expT[:, st, c * 512:(c + 1) * 512]
                            nc.scalar.activation(esl, ps[:], Exp, scale=SCALE)
                            nc.tensor.matmul(den_ps[c][:], ones_col[:], esl,
                                             start=(st == 0), stop=(st == SKT - 1))
                    for c in range(2):
                        rec = p4m.tile([1, 512], F32R, name="rec")
                        with nc.allow_low_precision("fp32r is 4-byte; feeds PE broadcast"):
                            nc.vector.reciprocal(rec[:], den_ps[c][:])
                        bc = psum.tile([P, 512], F32, name="ps_mm")
                        nc.tensor.matmul(bc[:], ones_row[:], rec[:], start=True, stop=True)
                        nc.vector.tensor_copy(recip_bc[:, c * 512:(c + 1) * 512], bc[:])

                # ---- Phase E: context ----
                with tc.tile_pool(name="ctxT", bufs=1) as ctx_pool:
                    ctxT = ctx_pool.tile([P, OT, SQ], F32R, name="ctxT")
                    with tc.tile_pool(name="p5s", bufs=2) as p5s:
                        for ot in range(OT):
                            vstrip = ld_strip(p5s, v_st, ot * P, P, "v_strip")
                            for c in range(2):
                                ps = psum.tile([P, 512], F32, name="ps_mm")
                                for st in range(SKT):
                                    nc.tensor.matmul(
                                        ps[:], vstrip[:, st, :], expT[:, st, c * 512:(c + 1) * 512],
                                        start=(st == 0), stop=(st == SKT - 1))
                                csl = ctxT[:, ot, c * 512:(c + 1) * 512]
                                nc.vector.tensor_mul(csl, ps[:], recip_bc[:, c * 512:(c + 1) * 512])
                                nc.vector.tensor_scalar_add(csl, csl, bias_sb["bv"][:, ot:ot + 1])

                    # ---- Phase F: output projection ----
                    with tc.tile_pool(name="p6s", bufs=2) as p6s, \
                         tc.tile_pool(name="p6o", bufs=2) as p6o:
                        for mt in range(OT):
                            wstrip = ld_strip(p6s, dram["wo"], mt * P, P, "wo_strip")
                            outt = p6o.tile([P, SQ], F32, name="outt")
                            for c in range(2):
                                ps = psum.tile([P, 512], F32, name="ps_mm")
                                for o in range(OT):
                                    nc.tensor.matmul(
                                        ps[:], wstrip[:, o, :], ctxT[:, o, c * 512:(c + 1) * 512],
                                        start=(o == 0), stop=(o == OT - 1))
                                nc.scalar.activation(outt[:, c * 512:(c + 1) * 512], ps[:],
                                                     Ident, bias=bias_sb["bo"][:, mt:mt + 1])
                            nc.sync.dma_start(out=outT[mt * P:(mt + 1) * P, :], in_=outt[:])
    return nc


def _build():
    global _PROG
    if _PROG is not None:
        return _PROG
    import concourse.bass as bass  # noqa: F401
    import concourse.mybir as mybir
    import concourse.tile as tile
    from concourse import bacc

    nc = bacc.Bacc("TRN2", target_bir_lowering=False, debug=False)
    _emit(nc, tile, mybir)
    nc.compile()
    _PROG = nc
    return nc


def _rope_tables():
    inv_freq = 1.0 / (10000.0 ** (np.arange(0, H, 2, dtype=np.float32) / H))
    t = np.arange(S, dtype=np.float32)
    freqs = np.outer(t, inv_freq).astype(np.float32)      # [S, H/2]
    cosT = np.ascontiguousarray(np.cos(freqs).T.astype(np.float16))  # [H/2, S]
    sinT = np.ascontiguousarray(np.sin(freqs).T.astype(np.float16))
    return cosT, sinT


def kernel(**inputs):
    nc = _build()
    from concourse.bass_utils import run_bass_kernel_spmd

    q = np.asarray(inputs["query"], dtype=np.float32)
    k = np.asarray(inputs["key"], dtype=np.float32)
    v = np.asarray(inputs["value"], dtype=np.float32)
    cosT, sinT = _rope_tables()
    wT = {n: np.ascontiguousarray(np.asarray(inputs[n], dtype=np.float32).T)
          for n in ("Wq", "Wk", "Wv", "Wo")}
    bias = {n: np.ascontiguousarray(np.asarray(inputs[n], dtype=np.float32))
            for n in ("bq", "bk", "bv", "bo")}
    ones_col = np.ones((P, 1), np.float32)
    ones_row = np.ones((1, P), np.float32)

    in_maps = []
    for c in range(NC_):
        b, half = divmod(c, 2)
        sl = slice(half * SQ, (half + 1) * SQ)
        in_maps.append({
            "qT": np.ascontiguousarray(q[b].T[:, sl]),
            "kT": np.ascontiguousarray(k[b].T),
            "vT": np.ascontiguousarray(v[b].T),
            "cos_q": np.ascontiguousarray(cosT[:, sl]),
            "sin_q": np.ascontiguousarray(sinT[:, sl]),
            "cos_k": cosT, "sin_k": sinT,
            "wq": wT["Wq"], "wk": wT["Wk"], "wv": wT["Wv"], "wo": wT["Wo"],
            "bq": bias["bq"], "bk": bias["bk"], "bv": bias["bv"], "bo": bias["bo"],
            "ones_col": ones_col, "ones_row": ones_row,
        })

    res = run_bass_kernel_spmd(nc, in_maps, core_ids=list(range(NC_)), trace=_TRACE)
    global LAST_RES
    LAST_RES = res
    out = np.empty((B, S, H), np.float32)
    for c in range(NC_):
        b, half = divmod(c, 2)
        out[b, half * SQ:(half + 1) * SQ, :] = res.results[c]["outT"].T
    return out


# revision 17
# speedup vs baseline: 1.2863x; 1.0503x over previous
"""Trainium2 Bass kernel for nn_CrossModalAttention (B=4, S=2048, H=2048, single head).

Sharding: 8 cores = 4 batches x 2 sequence halves. Each core computes the full
attention output for its (batch, query-half): it projects its 1024 query rows
and (duplicated across the pair) the full 2048 K/V rows of its batch element.

Device-side dataflow keeps every activation TRANSPOSED ([feature, seq]) so all
matmuls contract over the SBUF partition dim with no on-device transposes:

  A   v = value @ WvT   (WvT resident in 2 o-halves so the 2nd half's load
      hides under the 1st half's matmuls)  -> v staged to HBM
      + concurrently: Q-RoPE on DVE (3D tensor ops) -> qrotT staged to HBM
  B   kT' = (WkT.T @ krotT) + bk with K-RoPE fused inline per chunk (no HBM
      round-trip for krotT)               -> kT' staged to HBM
  C   qT' = (WqT.T @ qrotT) + bq  (WqT in 2 halves)  -> qT' staged to HBM
  D   expT = exp(scale * kT'.T @ qT') ; den = ones.T @ expT  (softmax without
      max-subtraction; scores are bounded ~|4| so exp cannot overflow)
  E   ctxT = (v.T-strips @ expT) * (1/den bcast) + bv   (softmax normalization
      and the V bias folded into the PSUM eviction; sum(probs)=1 makes the
      bias fold exact)
  F   outT = (WoT.T @ ctxT) + bo  -> HBM

All matmul operands are float32r (FP22 multiply, fp32 accumulate): full PE rate
at moving-dim >= 256 with ~2^-13 relative precision. RoPE tables are fp16.
"""

import sys

for _p in ("/opt/trn_rl_repo",):
    if _p not in sys.path:
        sys.path.append(_p)

import numpy as np

B, S, H = 4, 2048, 2048
P = 128
HO = H // P            # 16 h-tiles
OT = H // P            # 16 o-tiles
SKT = S // P           # 16 key-position tiles
SQ = S // 2            # 1024 query rows per core
SK = S                 # 2048 key rows
NC_ = 8
SCALE = 1.0 / float(np.sqrt(H))

_PROG = None  # cached compiled program
_TRACE = False  # set True (with an NTFF hook installed) to profile the run
LAST_RES = None  # BassKernelResults of the most recent kernel() call


def _emit(nc, tile, mybir):
    F32 = mybir.dt.float32
    F16 = mybir.dt.float16
    F32R = mybir.dt.float32r
    Exp = mybir.ActivationFunctionType.Exp
    Ident = mybir.ActivationFunctionType.Identity

    dram = {}
    def din(name, shape, dt=F32):
        dram[name] = nc.dram_tensor(name, list(shape), dt, kind="ExternalInput").ap()
    din("qT", (H, SQ)); din("kT", (H, SK))
    din("vT", (H, SK), F32R)
    din("cos_q", (H // 2, SQ), F16); din("sin_q", (H // 2, SQ), F16)
    din("cos_k", (H // 2, SK), F16); din("sin_k", (H // 2, SK), F16)
    din("wq", (H, H), F32R); din("wk", (H, H), F32R)
    din("wv", (H, H), F32R); din("wo", (H, H), F32R)
    din("bq", (H,)); din("bk", (H,)); din("bv", (H,)); din("bo", (H,))
    din("ones_col", (P, 1), F32R); din("ones_row", (1, P), F32R)
    outT = nc.dram_tensor("outT", [H, SQ], F32, kind="ExternalOutput").ap()

    def strip_ap(src2d, col0, width):
        return src2d[:, col0:col0 + width].rearrange("(o p) s -> p o s", p=P)

    def ld_strip(pool, src2d, col0, width, tag):
        rows = src2d.shape[0]
        t = pool.tile([P, rows // P, width], src2d.dtype, name=tag)
        nc.sync.dma_start(out=t[:], in_=strip_ap(src2d, col0, width))
        return t

    def rope_chunk(pool, xsrc, cos_ap, sin_ap, c0, cw, tag):
        """Load [H, cw] chunk of xsrc + tables, return roped [P, HO, cw] f32r tile.

        rot_half: lo half uses x_lo*cos - x_hi*sin; hi half x_hi*cos + x_lo*sin.
        """
        HH = HO // 2
        x = pool.tile([P, HO, cw], F32, name=f"{tag}_in")
        nc.sync.dma_start(out=x[:], in_=strip_ap(xsrc, c0, cw))
        cs = pool.tile([P, HH, cw], F16, name=f"{tag}_cos")
        nc.sync.dma_start(out=cs[:], in_=strip_ap(cos_ap, c0, cw))
        sn = pool.tile([P, HH, cw], F16, name=f"{tag}_sin")
        nc.sync.dma_start(out=sn[:], in_=strip_ap(sin_ap, c0, cw))
        r = pool.tile([P, HO, cw], F32R, name=f"{tag}_out")
        tmp = pool.tile([P, HH, cw], F32, name=f"{tag}_tmp")
        lo, hi = slice(0, HH), slice(HH, HO)
        nc.vector.tensor_mul(r[:, lo, :], x[:, lo, :], cs[:])
        nc.vector.tensor_mul(tmp[:], x[:, hi, :], sn[:])
        nc.vector.tensor_sub(r[:, lo, :], r[:, lo, :], tmp[:])
        nc.vector.tensor_mul(r[:, hi, :], x[:, hi, :], cs[:])
        nc.vector.tensor_mul(tmp[:], x[:, lo, :], sn[:])
        nc.vector.tensor_add(r[:, hi, :], r[:, hi, :], tmp[:])
        return r

    with tile.TileContext(nc) as tc:
        from contextlib import ExitStack
        with ExitStack() as ctx:
            glob = ctx.enter_context(tc.tile_pool(name="glob", bufs=1))
            dstage = ctx.enter_context(tc.tile_pool(name="dram", bufs=1, space="DRAM"))
            psum = ctx.enter_context(tc.tile_pool(name="psum", bufs=5, space="PSUM"))

            # staging buffers in HBM
            qrot_st = dstage.tile([H, SQ], F32R, name="qrot_st")
            v_st = dstage.tile([SK, H], F32R, name="v_st")
            kTp_st = dstage.tile([H, SK], F32R, name="kTp_st")
            qTp_st = dstage.tile([H, SQ], F32R, name="qTp_st")

            # ---- Phase A: v-proj (wv halves) + q-rope staging, concurrent ----
            with tc.tile_pool(name="wvh0", bufs=1) as wvh0p, \
                 tc.tile_pool(name="wvh1", bufs=1) as wvh1p, \
                 tc.tile_pool(name="p1s", bufs=2) as p1s, \
                 tc.tile_pool(name="p1e", bufs=3) as p1e, \
                 tc.tile_pool(name="qrope", bufs=2) as qrope:
                wv_h = [wvh0p.tile([P, HO, H // 2], F32R, name="wv_h0"),
                        wvh1p.tile([P, HO, H // 2], F32R, name="wv_h1")]
                for hf in range(2):
                    nc.sync.dma_start(out=wv_h[hf][:],
                                      in_=strip_ap(dram["wv"], hf * (H // 2), H // 2))

                # biases as [P, 16] + ones helpers
                bias_sb = {}
                for bn in ("bq", "bk", "bv", "bo"):
                    bt = glob.tile([P, OT], F32, name=f"{bn}_sb")
                    nc.sync.dma_start(out=bt[:], in_=dram[bn].rearrange("(t p) -> p t", p=P))
                    bias_sb[bn] = bt
                ones_col = glob.tile([P, 1], F32R, name="ones_col")
                nc.sync.dma_start(out=ones_col[:], in_=dram["ones_col"])
                ones_row = glob.tile([1, P], F32R, name="ones_row")
                nc.sync.dma_start(out=ones_row[:], in_=dram["ones_row"])
                recip_bc = glob.tile([P, SQ], F32, name="recip_bc")

                for hf in range(2):
                    for st in range(SKT):
                        vstrip = ld_strip(p1s, dram["vT"], st * P, P, "vT_strip")
                        for oc in range(2):
                            ps = psum.tile([P, 512], F32, name="ps_mm")
                            for h in range(HO):
                                nc.tensor.matmul(
                                    ps[:], vstrip[:, h, :],
                                    wv_h[hf][:, h, oc * 512:(oc + 1) * 512],
                                    start=(h == 0), stop=(h == HO - 1))
                            ev = p1e.tile([P, 512], F32R, name="p1_ev")
                            nc.scalar.copy(ev[:], ps[:])
                            nc.sync.dma_start(
                                out=v_st[st * P:(st + 1) * P,
                                         (hf * 2 + oc) * 512:(hf * 2 + oc + 1) * 512],
                                in_=ev[:])

                # q-rope -> qrot_st (DVE work; overlaps the PE-heavy loop above)
                for c0 in range(0, SQ, 128):
                    r = rope_chunk(qrope, dram["qT"], dram["cos_q"], dram["sin_q"],
                                   c0, 128, "qr")
                    nc.sync.dma_start(out=strip_ap(qrot_st, c0, 128), in_=r[:])

            # ---- Phase B: k-proj with fused k-rope (WkT in 4 col-quarters so
            # the first quarter's matmuls start ~11us after space release) ----
            CW2 = 256
            with tc.tile_pool(name="p2w0", bufs=1) as pw20, \
                 tc.tile_pool(name="p2w1", bufs=1) as pw21, \
                 tc.tile_pool(name="p2w2", bufs=1) as pw22, \
                 tc.tile_pool(name="p2w3", bufs=1) as pw23, \
                 tc.tile_pool(name="p2in", bufs=1) as p2in, \
                 tc.tile_pool(name="p2tab", bufs=1) as p2tab, \
                 tc.tile_pool(name="p2r", bufs=2) as p2r, \
                 tc.tile_pool(name="p2e", bufs=3) as p2e:
                wk_q = []
                for qt, pw in enumerate((pw20, pw21, pw22, pw23)):
                    wt = pw.tile([P, HO, 512], F32R, name=f"wk_q{qt}")
                    nc.sync.dma_start(out=wt[:], in_=strip_ap(dram["wk"], qt * 512, 512))
                    wk_q.append(wt)
                for c0 in range(0, SK, CW2):
                    HH = HO // 2
                    x = p2in.tile([P, HO, CW2], F32, name="kr_in")
                    nc.sync.dma_start(out=x[:], in_=strip_ap(dram["kT"], c0, CW2))
                    cs = p2tab.tile([P, HH, CW2], F16, name="kr_cos")
                    nc.sync.dma_start(out=cs[:], in_=strip_ap(dram["cos_k"], c0, CW2))
                    sn = p2tab.tile([P, HH, CW2], F16, name="kr_sin")
                    nc.sync.dma_start(out=sn[:], in_=strip_ap(dram["sin_k"], c0, CW2))
                    r = p2r.tile([P, HO, CW2], F32R, name="kr_out")
                    tmp = p2r.tile([P, HH, CW2], F32, name="kr_tmp")
                    lo, hi = slice(0, HH), slice(HH, HO)
                    nc.vector.tensor_mul(r[:, lo, :], x[:, lo, :], cs[:])
                    nc.vector.tensor_mul(tmp[:], x[:, hi, :], sn[:])
                    nc.vector.tensor_sub(r[:, lo, :], r[:, lo, :], tmp[:])
                    nc.vector.tensor_mul(r[:, hi, :], x[:, hi, :], cs[:])
                    nc.vector.tensor_mul(tmp[:], x[:, lo, :], sn[:])
                    nc.vector.tensor_add(r[:, hi, :], r[:, hi, :], tmp[:])
                    for qt in range(4):
                        for otl in range(4):
                            ot = qt * 4 + otl
                            ps = psum.tile([P, 512], F32, name="ps_mm")[:, :CW2]
                            for h in range(HO):
                                nc.tensor.matmul(
                                    ps, wk_q[qt][:, h, otl * P:(otl + 1) * P], r[:, h, :],
                                    start=(h == 0), stop=(h == HO - 1))
                            ev = p2e.tile([P, CW2], F32R, name="p2_ev")
                            nc.scalar.activation(ev[:], ps, Ident, bias=bias_sb["bk"][:, ot:ot + 1])
                            nc.sync.dma_start(
                                out=kTp_st[ot * P:(ot + 1) * P, c0:c0 + CW2], in_=ev[:])

            # ---- Phase C: q-proj (wq in 4 col-quarters; each qrot chunk read once) ----
            with tc.tile_pool(name="wq0", bufs=1) as wq0p, \
                 tc.tile_pool(name="wq1", bufs=1) as wq1p, \
                 tc.tile_pool(name="wq2", bufs=1) as wq2p, \
                 tc.tile_pool(name="wq3", bufs=1) as wq3p, \
                 tc.tile_pool(name="p3in", bufs=2) as p3in, \
                 tc.tile_pool(name="p3e", bufs=3) as p3e:
                wq_q = []
                for qt, pw in enumerate((wq0p, wq1p, wq2p, wq3p)):
                    wt = pw.tile([P, HO, 512], F32R, name=f"wq_q{qt}")
                    nc.sync.dma_start(out=wt[:], in_=strip_ap(dram["wq"], qt * 512, 512))
                    wq_q.append(wt)
                for c0 in range(0, SQ, 512):
                    rch = p3in.tile([P, HO, 512], F32R, name="p3_rch")
                    nc.sync.dma_start(out=rch[:], in_=strip_ap(qrot_st, c0, 512))
                    for qt in range(4):
                        for otl in range(4):
                            ot = qt * 4 + otl
                            ps = psum.tile([P, 512], F32, name="ps_mm")
                            for h in range(HO):
                                nc.tensor.matmul(
                                    ps[:], wq_q[qt][:, h, otl * P:(otl + 1) * P],
                                    rch[:, h, :],
                                    start=(h == 0), stop=(h == HO - 1))
                            ev = p3e.tile([P, 512], F32R, name="p3_ev")
                            nc.scalar.activation(ev[:], ps[:], Ident,
                                                 bias=bias_sb["bq"][:, ot:ot + 1])
                            nc.sync.dma_start(
                                out=qTp_st[ot * P:(ot + 1) * P, c0:c0 + 512], in_=ev[:])

            # ---- Phase D: scores.T -> exp -> den ----
            with tc.tile_pool(name="expT", bufs=1) as expT_pool:
                expT = expT_pool.tile([P, SKT, SQ], F32R, name="expT")
                with tc.tile_pool(name="p4q", bufs=1) as p4q, \
                     tc.tile_pool(name="p4s", bufs=3) as p4s, \
                     tc.tile_pool(name="p4den", bufs=1, space="PSUM") as p4den, \
                     tc.tile_pool(name="p4m", bufs=2) as p4m:
                    qTp_sb = p4q.tile([P, OT, SQ], F32R, name="qTp_sb")
                    nc.sync.dma_start(out=qTp_sb[:], in_=qTp_st.rearrange("(o p) s -> p o s", p=P))
                    _den = p4den.tile([1, 1024], F32, name="den")
                    den_ps = [_den[:, 0:512], _den[:, 512:1024]]
                    for st in range(SKT):
                        kstrip = ld_strip(p4s, kTp_st, st * P, P, "kTp_strip")
                        for c in range(2):
                            ps = psum.tile([P, 512], F32, name="ps_mm")
                            for o in range(OT):
                                nc.tensor.matmul(
                                    ps[:], kstrip[:, o, :], qTp_sb[:, o, c * 512:(c + 1) * 512],
                                    start=(o == 0), stop=(o == OT - 1))
                            esl = expT[:, st, c * 512:(c + 1) * 512]
                            nc.scalar.activation(esl, ps[:], Exp, scale=SCALE)
                            nc.tensor.matmul(den_ps[c][:], ones_col[:], esl,
                                             start=(st == 0), stop=(st == SKT - 1))
                    for c in range(2):
                        rec = p4m.tile([1, 512], F32R, name="rec")
                        with nc.allow_low_precision("fp32r is 4-byte; feeds PE broadcast"):
                            nc.vector.reciprocal(rec[:], den_ps[c][:])
                        bc = psum.tile([P, 512], F32, name="ps_mm")
                        nc.tensor.matmul(bc[:], ones_row[:], rec[:], start=True, stop=True)
                        nc.vector.tensor_copy(recip_bc[:, c * 512:(c + 1) * 512], bc[:])

                # ---- Phase E: context ----
                with tc.tile_pool(name="ctxT", bufs=1) as ctx_pool:
                    ctxT = ctx_pool.tile([P, OT, SQ], F32R, name="ctxT")
                    with tc.tile_pool(name="p5s", bufs=3) as p5s:
                        for ot in range(OT):
                            vstrip = ld_strip(p5s, v_st, ot * P, P, "v_strip")
                            for c in range(2):
                                ps = psum.tile([P, 512], F32, name="ps_mm")
                                for st in range(SKT):
                                    nc.tensor.matmul(
                                        ps[:], vstrip[:, st, :], expT[:, st, c * 512:(c + 1) * 512],
                                        start=(st == 0), stop=(st == SKT - 1))
                                csl = ctxT[:, ot, c * 512:(c + 1) * 512]
                                nc.vector.tensor_mul(csl, ps[:], recip_bc[:, c * 512:(c + 1) * 512])
                                nc.vector.tensor_scalar_add(csl, csl, bias_sb["bv"][:, ot:ot + 1])

                    # ---- Phase F: output projection ----
                    with tc.tile_pool(name="p6s", bufs=3) as p6s, \
                         tc.tile_pool(name="p6o", bufs=2) as p6o:
                        for mt in range(OT):
                            wstrip = ld_strip(p6s, dram["wo"], mt * P, P, "wo_strip")
                            outt = p6o.tile([P, SQ], F32, name="outt")
                            for c in range(2):
                                ps = psum.tile([P, 512], F32, name="ps_mm")
                                for o in range(OT):
                                    nc.tensor.matmul(
                                        ps[:], wstrip[:, o, :], ctxT[:, o, c * 512:(c + 1) * 512],
                                        start=(o == 0), stop=(o == OT - 1))
                                nc.scalar.activation(outt[:, c * 512:(c + 1) * 512], ps[:],
                                                     Ident, bias=bias_sb["bo"][:, mt:mt + 1])
                            nc.sync.dma_start(out=outT[mt * P:(mt + 1) * P, :], in_=outt[:])
    return nc


def _build():
    global _PROG
    if _PROG is not None:
        return _PROG
    import concourse.bass as bass  # noqa: F401
    import concourse.mybir as mybir
    import concourse.tile as tile
    from concourse import bacc

    nc = bacc.Bacc("TRN2", target_bir_lowering=False, debug=False)
    _emit(nc, tile, mybir)
    nc.compile()
    _PROG = nc
    return nc


def _rope_tables():
    inv_freq = 1.0 / (10000.0 ** (np.arange(0, H, 2, dtype=np.float32) / H))
    t = np.arange(S, dtype=np.float32)
    freqs = np.outer(t, inv_freq).astype(np.float32)      # [S, H/2]
    cosT = np.ascontiguousarray(np.cos(freqs).T.astype(np.float16))  # [H/2, S]
    sinT = np.ascontiguousarray(np.sin(freqs).T.astype(np.float16))
    return cosT, sinT


def kernel(**inputs):
    nc = _build()
    from concourse.bass_utils import run_bass_kernel_spmd

    q = np.asarray(inputs["query"], dtype=np.float32)
    k = np.asarray(inputs["key"], dtype=np.float32)
    v = np.asarray(inputs["value"], dtype=np.float32)
    cosT, sinT = _rope_tables()
    wT = {n: np.ascontiguousarray(np.asarray(inputs[n], dtype=np.float32).T)
          for n in ("Wq", "Wk", "Wv", "Wo")}
    bias = {n: np.ascontiguousarray(np.asarray(inputs[n], dtype=np.float32))
            for n in ("bq", "bk", "bv", "bo")}
    ones_col = np.ones((P, 1), np.float32)
    ones_row = np.ones((1, P), np.float32)

    in_maps = []
    for c in range(NC_):
        b, half = divmod(c, 2)
        sl = slice(half * SQ, (half + 1) * SQ)
        in_maps.append({
            "qT": np.ascontiguousarray(q[b].T[:, sl]),
            "kT": np.ascontiguousarray(k[b].T),
            "vT": np.ascontiguousarray(v[b].T),
            "cos_q": np.ascontiguousarray(cosT[:, sl]),
            "sin_q": np.ascontiguousarray(sinT[:, sl]),
            "cos_k": cosT, "sin_k": sinT,
            "wq": wT["Wq"], "wk": wT["Wk"], "wv": wT["Wv"], "wo": wT["Wo"],
            "bq": bias["bq"], "bk": bias["bk"], "bv": bias["bv"], "bo": bias["bo"],
            "ones_col": ones_col, "ones_row": ones_row,
        })

    res = run_bass_kernel_spmd(nc, in_maps, core_ids=list(range(NC_)), trace=_TRACE)
    global LAST_RES
    LAST_RES = res
    out = np.empty((B, S, H), np.float32)
    for c in range(NC_):
        b, half = divmod(c, 2)
        out[b, half * SQ:(half + 1) * SQ, :] = res.results[c]["outT"].T
    return out
